# revision 21
# baseline (speedup 1.0000x reference)
"""GCN layer (message passing) on 8 Trainium2 NeuronCores.

out = relu(((D^-1/2 A D^-1/2) X) @ W.T) + X

Strategy (dst-sharded graph partitioning, bf16 gather table):
  - Destination nodes sharded across 8 cores (12500 each); every core holds
    the full feature table and computes its 12500 output rows; host concats.
  - Device prologue: cast the f32 feature table to a bf16 DRAM table h with
    the pre-norm D^-1/2 folded in (h[n] = norm[n] * x[n]); partition p casts
    16 consecutive rows per iteration so DMA descriptors stay contiguous.
    The post-norm norm[dst] is folded into the final ReLU's per-partition
    scale, so the one-hot scatter matrices are pure 0/1.
  - Main loop over PAIRS of dst tiles (2x128 nodes): 4 dma_gather calls per
    pair (one per src bucket of 25088 nodes, int16 indices) pull the edge
    source rows as bf16 into a shared X [128 slots, Cp*128]. Pairing halves
    the SWDGE DMA count so the tile framework's 8 DMASW completion-sem
    lanes recycle every ~4 tiles instead of 2 - the lane-reuse wait (full
    DMA completion of the gather 8 back) stops gating the Pool engine.
    Edges are sorted (pair, bucket, tile, src); the chunk straddling the
    two tiles' boundary is fed to BOTH tiles' matmuls, with the per-tile
    one-hot ld arrays set to -1 for the other tile's slots.
  - Per tile: the one-hot S [128, nct*128] bf16 is built in ONE wide DVE
    tensor_tensor (iota broadcast along chunks, ld broadcast along the 128
    lane dim, via stride-0 APs), then nct bf16 matmuls accumulate zT[i,d]
    in PSUM f32, y = relu(norm_dst * (zT.T @ W.T)) on ACT, residual on DVE.
  - Unwritten X slots (cross-core count spread + chunk padding) are
    memzeroed so NaN garbage can't poison the 0-weighted matmul lanes.
    Idx streams are padded with 0 (gathers bucket row 0; killed by ld=-1).
    (-1 idx padding, which the gather ucode strips, crashes the device.)
"""

import math

import numpy as np

import concourse.bacc as bacc
import concourse.mybir as mybir
from concourse.bass import AP
from concourse.bass_utils import run_bass_kernel_spmd
from concourse.tile import TileContext

P = 128
N_CORES = 8
NB = 4
B = 25088  # bucket size (multiple of 128, int16-indexable)
NPAD = NB * B  # padded node count 100352
CAST_G = 16  # rows per partition per cast iteration
N_NODES = 100000
SINGLE_PACKET = False


def _prepare(features, W, edge_src, edge_dst, n_cores=N_CORES):
    features = np.asarray(features, dtype=np.float32)
    W = np.asarray(W, dtype=np.float32)
    edge_src = np.asarray(edge_src, dtype=np.int32)
    edge_dst = np.asarray(edge_dst, dtype=np.int32)

    n_nodes, d = features.shape
    assert d == P and n_nodes == N_NODES
    npc = n_nodes // n_cores
    n_tiles = math.ceil(npc / P)
    assert n_tiles % 2 == 0
    n_pairs = n_tiles // 2
    rows_last = npc - (n_tiles - 1) * P

    degs = np.bincount(edge_dst, minlength=n_nodes).astype(np.float32)
    norm = 1.0 / np.sqrt(np.maximum(degs, 1.0), dtype=np.float32)
    norm_pad = np.zeros(NPAD, np.float32)
    norm_pad[:n_nodes] = norm

    featspad = np.zeros((NPAD, P), np.float32)
    featspad[:n_nodes] = features

    # normP[p, j*CAST_G + g] = norm[j*128*CAST_G + p*CAST_G + g]
    n_cast_cols = NPAD // P  # 784
    normP = norm_pad.reshape(n_cast_cols // CAST_G, P, CAST_G)
    normP = np.ascontiguousarray(normP.transpose(1, 0, 2).reshape(P, n_cast_cols))

    core_of = edge_dst // npc

    # per-core sorted edge lists, (pair, bucket) counts, even-tile splits
    per_core = []
    counts_pb = np.zeros((n_cores, n_pairs, NB), np.int64)
    counts_even = np.zeros((n_cores, n_pairs, NB), np.int64)
    for k in range(n_cores):
        sel = np.flatnonzero(core_of == k)
        src_k = edge_src[sel]
        ldst = edge_dst[sel] - k * npc
        tile_of = ldst // P
        pair_of = tile_of // 2
        bucket = src_k // B
        order = np.lexsort((src_k, tile_of, bucket, pair_of))
        sel = sel[order]
        t_s = tile_of[order]
        gid = pair_of[order] * NB + bucket[order]
        counts_pb[k] = np.bincount(gid, minlength=n_pairs * NB).reshape(n_pairs, NB)
        counts_even[k] = np.bincount(
            gid[t_s % 2 == 0], minlength=n_pairs * NB
        ).reshape(n_pairs, NB)
        per_core.append((sel, gid, t_s, (ldst[order] % P).astype(np.float32)))

    n_pb = counts_pb.max(axis=0)  # static gather sizes [n_pairs, NB]
    assert n_pb.sum(axis=1).min() > 0
    ct_pb = (n_pb + P - 1) // P  # chunks per (pair, bucket)
    C_p = ct_pb.sum(axis=1)
    icols_pb = (n_pb + 15) // 16
    icols_p = icols_pb.max(axis=1)  # banded layout: buckets share columns

    chunk_off_in_pair = np.cumsum(ct_pb, axis=1) - ct_pb
    icol_off_pair = np.concatenate([[0], np.cumsum(icols_p)])[:-1]
    total_icols = int(icols_p.sum())

    # per-tile chunk ranges within the pair's chunk space (static = cross-core
    # envelope): even tile owns bucket chunks [0, e_end); odd [o_start, cb)
    s_max = counts_even.max(axis=0)
    s_min = counts_even.min(axis=0)
    e_end = (s_max + P - 1) // P  # [n_pairs, NB]
    e_end = np.minimum(e_end, ct_pb)
    o_start = s_min // P
    o_start = np.minimum(o_start, ct_pb)  # empty-odd guard
    nct_even = e_end.sum(axis=1)
    nct_odd = (ct_pb - o_start).sum(axis=1)
    nct_t = np.zeros(n_tiles, np.int64)
    nct_t[0::2] = nct_even
    nct_t[1::2] = nct_odd
    assert nct_t.min() > 0
    ld_col_off = np.concatenate([[0], np.cumsum(nct_t)])[:-1]
    total_C = int(nct_t.sum())

    # chunk lists per tile: pair-chunk index for each S column
    chunk_lists = []
    for t in range(n_tiles):
        p, half = t // 2, t % 2
        lst = []
        for b in range(NB):
            co = int(chunk_off_in_pair[p, b])
            if half == 0:
                lst.extend(range(co, co + int(e_end[p, b])))
            else:
                lst.extend(range(co + int(o_start[p, b]), co + int(ct_pb[p, b])))
        chunk_lists.append(lst)
        assert len(lst) == nct_t[t]

    layout = dict(
        npc=npc,
        n_tiles=n_tiles,
        n_pairs=n_pairs,
        rows_last=rows_last,
        n_pb=n_pb,
        ct_pb=ct_pb,
        C_p=C_p,
        icols_pb=icols_pb,
        icols_p=icols_p,
        chunk_off_in_pair=chunk_off_in_pair,
        icol_off_pair=icol_off_pair,
        total_icols=total_icols,
        nct_t=nct_t,
        ld_col_off=ld_col_off,
        total_C=total_C,
        chunk_lists=chunk_lists,
        n_cast_cols=n_cast_cols,
    )

    ecol_off = np.concatenate(
        [np.zeros((n_pairs, 1), np.int64), np.cumsum(e_end, axis=1)[:, :-1]], axis=1
    )
    ocol_off = np.concatenate(
        [np.zeros((n_pairs, 1), np.int64), np.cumsum(ct_pb - o_start, axis=1)[:, :-1]],
        axis=1,
    )

    in_maps = []
    wt = np.ascontiguousarray(W.T)
    iotam = np.tile(np.arange(P, dtype=np.float32), (P, 1))
    for k in range(n_cores):
        sel, gid, t_s, ld_sorted = per_core[k]
        group_start = np.zeros(n_pairs * NB, np.int64)
        cnts = counts_pb[k].reshape(-1)
        group_start[1:] = np.cumsum(cnts)[:-1]
        pos = np.arange(len(sel)) - group_start[gid]
        p_of = gid // NB
        b_of = gid % NB

        # pad with 0 (gathers bucket row 0; killed by ld=-1 in S).
        # banded: bucket b's stream lives in partitions [32b, 32b+32)
        # (queue b's Q7 core pair), replicated twice within the band.
        idx16 = np.zeros((NB, 16, total_icols), np.int16)
        icol = icol_off_pair[p_of] + pos // 16
        idx16[b_of, pos % 16, icol] = (edge_src[sel] - b_of * B).astype(np.int16)
        idxm = np.concatenate([np.tile(idx16[b], (2, 1)) for b in range(NB)], axis=0)

        # ld array [128, total_C]: tile t's columns are its chunk list; an
        # edge of tile t in pair-chunk (relative) c lands at the column where
        # t's list contains c (straddle chunks appear in both tiles' lists;
        # each edge is written only into its own tile's column)
        import ml_dtypes
        ldm = np.full((P, total_C), -1.0, np.float32)
        rel_chunk = pos // P
        half = t_s % 2
        col_even = ld_col_off[2 * p_of] + ecol_off[p_of, b_of] + rel_chunk
        col_odd = (
            ld_col_off[2 * p_of + 1]
            + ocol_off[p_of, b_of]
            + rel_chunk
            - o_start[p_of, b_of]
        )
        col = np.where(half == 0, col_even, col_odd)
        assert (rel_chunk[half == 0] < e_end[p_of, b_of][half == 0]).all()
        assert (rel_chunk[half == 1] >= o_start[p_of, b_of][half == 1]).all()
        ldm[pos % P, col] = ld_sorted
        ldm = ldm.astype(ml_dtypes.bfloat16)

        # normcol[p, t] = norm[k*npc + t*128 + p] (own dst rows)
        nslice = np.zeros(n_tiles * P, np.float32)
        nslice[:npc] = norm[k * npc : (k + 1) * npc]
        normcol = np.ascontiguousarray(nslice.reshape(n_tiles, P).T)

        in_maps.append(
            {
                "featspad": featspad,
                "idxm": np.ascontiguousarray(idxm),
                "ldm": np.ascontiguousarray(ldm),
                "wt": wt,
                "iotam": iotam,
                "normP": normP,
                "normcol": normcol,
                "resid": np.ascontiguousarray(features[k * npc : (k + 1) * npc]),
            }
        )
    return in_maps, layout


def _build_program(layout):
    f32 = mybir.dt.float32
    bf16 = mybir.dt.bfloat16
    fp8 = mybir.dt.float8e4
    i16 = mybir.dt.int16
    npc = layout["npc"]
    n_tiles = layout["n_tiles"]
    n_pairs = layout["n_pairs"]
    rows_last = layout["rows_last"]
    n_pb = layout["n_pb"]
    ct_pb = layout["ct_pb"]
    C_p = layout["C_p"]
    icols_pb = layout["icols_pb"]
    icols_p = layout["icols_p"]
    chunk_off_in_pair = layout["chunk_off_in_pair"]
    icol_off_pair = layout["icol_off_pair"]
    nct_t = layout["nct_t"]
    ld_col_off = layout["ld_col_off"]
    chunk_lists = layout["chunk_lists"]
    n_cast_cols = layout["n_cast_cols"]
    Cmax = int(C_p.max())
    nct_max = int(nct_t.max())

    nc = bacc.Bacc(num_swdge_queues=4, dynamic_dma_scratch_size=32768)
    featspad = nc.declare_dram_parameter("featspad", [NPAD, P], f32, isOutput=False)
    idxm = nc.declare_dram_parameter(
        "idxm", [P, layout["total_icols"]], i16, isOutput=False
    )
    ldm = nc.declare_dram_parameter(
        "ldm", [P, layout["total_C"]], bf16, isOutput=False
    )
    wt = nc.declare_dram_parameter("wt", [P, P], f32, isOutput=False)
    iotam = nc.declare_dram_parameter("iotam", [P, P], f32, isOutput=False)
    normP = nc.declare_dram_parameter("normP", [P, n_cast_cols], f32, isOutput=False)
    normcol = nc.declare_dram_parameter("normcol", [P, n_tiles], f32, isOutput=False)
    resid = nc.declare_dram_parameter("resid", [npc, P], f32, isOutput=False)
    out = nc.declare_dram_parameter("out", [npc, P], bf16, isOutput=True)

    with TileContext(nc) as tc:
        with (
            tc.tile_pool(name="const", bufs=1) as constp,
            tc.tile_pool(name="hdram", bufs=1, space="DRAM") as hp,
            tc.tile_pool(name="cast", bufs=4) as castp,
            tc.tile_pool(name="meta", bufs=3) as metap,
            tc.tile_pool(name="x", bufs=4) as xp,
            tc.tile_pool(name="s", bufs=3) as sp,
            tc.tile_pool(name="zps", bufs=2, space="PSUM") as zpsp,
            tc.tile_pool(name="yps", bufs=2, space="PSUM") as ypsp,
            tc.tile_pool(name="post", bufs=3) as postp,
        ):
            wt_f = constp.tile([P, P], f32)
            nc.sync.dma_start(out=wt_f[:], in_=wt[:, :])
            wt_bf = constp.tile([P, P], bf16)
            nc.scalar.copy(out=wt_bf[:], in_=wt_f[:])
            iota_f = constp.tile([P, P], f32)
            nc.sync.dma_start(out=iota_f[:], in_=iotam[:, :])
            iota_bf = constp.tile([P, P], bf16)
            nc.scalar.copy(out=iota_bf[:], in_=iota_f[:])
            normP_sb = constp.tile([P, n_cast_cols], f32)
            nc.sync.dma_start(out=normP_sb[:], in_=normP[:, :])
            normcol_sb = constp.tile([P, n_tiles], f32)
            nc.sync.dma_start(out=normcol_sb[:], in_=normcol[:, :])

            hfull = hp.tile([NPAD, P], bf16)

            # pre-zero the X pool's rotation slots once: later pairs' unwritten
            # slots then hold zeros or stale finite bf16 from 4 pairs back,
            # which the 0-weighted (ld=-1) one-hot lanes annihilate - NaN-free
            # without per-bucket memzeros on the gather critical path.
            for _ in range(4):
                X_zero = xp.tile([P, Cmax * P], bf16, tag="X")
                nc.scalar.memzero(X_zero[:])

            # ---- prologue: h = norm * x, f32 -> bf16, 2048 rows/iter ----
            for j in range(n_cast_cols // CAST_G):
                r0 = j * P * CAST_G
                cin = castp.tile([P, CAST_G * P], f32, tag="cin")
                nc.scalar.dma_start(
                    out=cin[:].rearrange("p (g d) -> p g d", d=P),
                    in_=featspad[r0 : r0 + P * CAST_G, :].rearrange(
                        "(p g) d -> p g d", g=CAST_G
                    ),
                )
                cout = castp.tile([P, CAST_G * P], bf16, tag="cout")
                nrm = normP_sb[:, j * CAST_G : (j + 1) * CAST_G]
                nc.vector.scalar_tensor_tensor(
                    out=cout[:].rearrange("p (g d) -> p g d", d=P),
                    in0=cin[:].rearrange("p (g d) -> p g d", d=P),
                    scalar=1.0,
                    in1=AP(nrm.tensor, nrm.offset, [nrm.ap[0], [1, CAST_G], [0, P]]),
                    op0=mybir.AluOpType.mult,
                    op1=mybir.AluOpType.mult,
                )
                nc.scalar.dma_start(
                    out=hfull[r0 : r0 + P * CAST_G, :].rearrange(
                        "(p g) d -> p g d", g=CAST_G
                    ),
                    in_=cout[:].rearrange("p (g d) -> p g d", d=P),
                )

            # ---- main loop over pairs of dst tiles ----
            for p in range(n_pairs):
                icols = int(icols_p[p])
                mt_i = metap.tile([P, max(icols, 1)], i16, tag="mi")
                ic0 = int(icol_off_pair[p])
                nc.sync.dma_start(out=mt_i[:, :icols], in_=idxm[:, ic0 : ic0 + icols])

                Cp = int(C_p[p])
                X_full = xp.tile([P, Cmax * P], bf16, tag="X")
                X = X_full[:, : Cp * P]
                for b in range(NB):
                    n_idx = int(n_pb[p, b])
                    if n_idx == 0:
                        continue
                    co = int(chunk_off_in_pair[p, b])
                    cb = int(ct_pb[p, b])
                    io = 0
                    icb = int(icols_pb[p, b])
                    nc.gpsimd.dma_gather(
                        out_ap=X[:, co * P : (co + cb) * P].rearrange(
                            "p (c e) -> p c e", e=P
                        ),
                        in_ap=hfull[b * B : (b + 1) * B, :],
                        idxs_ap=mt_i[:, io : io + icb],
                        num_idxs=n_idx,
                        num_idxs_reg=n_idx,
                        elem_size=P,
                        single_packet=SINGLE_PACKET,
                        queue_num=b % 2,
                    )

                for half in range(2):
                    t = 2 * p + half
                    nct = int(nct_t[t])
                    lc0 = int(ld_col_off[t])
                    ld_bf = metap.tile([P, nct_max], bf16, tag="mldb")
                    nc.sync.dma_start(out=ld_bf[:, :nct], in_=ldm[:, lc0 : lc0 + nct])

                    # wide 0/1 one-hot: S[e, (j,d)] = (iota[d] == ld[e,j])
                    S = sp.tile([P, nct_max * P], bf16, tag="S")
                    nc.vector.tensor_tensor(
                        out=S[:, : nct * P].rearrange("p (c d) -> p c d", d=P),
                        in0=AP(
                            iota_bf.tensor,
                            iota_bf.offset,
                            [iota_bf.ap[0], [0, nct], [1, P]],
                        ),
                        in1=AP(
                            ld_bf.tensor, ld_bf.offset, [ld_bf.ap[0], [1, nct], [0, P]]
                        ),
                        op=mybir.AluOpType.is_equal,
                    )

                    z_ps = zpsp.tile([P, P], f32)
                    for j, c in enumerate(chunk_lists[t]):
                        # zT[i, d] += X_c[e, i].T @ S_j[e, d]
                        nc.tensor.matmul(
                            out=z_ps[:],
                            lhsT=X[:, c * P : (c + 1) * P],
                            rhs=S[:, j * P : (j + 1) * P],
                            start=(j == 0),
                            stop=(j == nct - 1),
                        )

                    zT_bf = postp.tile([P, P], bf16, tag="zT")
                    nc.scalar.copy(out=zT_bf[:], in_=z_ps[:])
                    y_ps = ypsp.tile([P, P], f32)
                    # y[d, o] = zT[i, d].T @ wt[i, o]
                    nc.tensor.matmul(
                        out=y_ps[:], lhsT=zT_bf[:], rhs=wt_bf[:], start=True, stop=True
                    )

                    rows = P if t < n_tiles - 1 else rows_last
                    y_sb = postp.tile([P, P], f32, tag="y")
                    # relu(norm_dst * y): post-norm folded into the scale
                    nc.scalar.activation(
                        out=y_sb[:],
                        in_=y_ps[:],
                        func=mybir.ActivationFunctionType.Relu,
                        scale=normcol_sb[:, t : t + 1],
                    )
                    res_sb = postp.tile([P, P], f32, tag="res")
                    nc.sync.dma_start(
                        out=res_sb[:rows], in_=resid[t * P : t * P + rows, :]
                    )
                    o_sb = postp.tile([P, P], bf16, tag="o")
                    nc.vector.tensor_add(
                        out=o_sb[:rows], in0=y_sb[:rows], in1=res_sb[:rows]
                    )
                    nc.sync.dma_start(
                        out=out[t * P : t * P + rows, :], in_=o_sb[:rows]
                    )
    nc.finalize()
    return nc


def _run(features, W, edge_src, edge_dst, trace=False, **spmd_kwargs):
    in_maps, layout = _prepare(features, W, edge_src, edge_dst)
    nc = _build_program(layout)
    br = run_bass_kernel_spmd(
        nc, in_maps, core_ids=list(range(N_CORES)), trace=trace, **spmd_kwargs
    )
    outs = [np.asarray(r["out"]).astype(np.float32) for r in br.results]
    full = np.concatenate(outs, axis=0)
    return full, br


def kernel(features, W, edge_src, edge_dst):
    out, _ = _run(features, W, edge_src, edge_dst, trace=False)
    return out


# revision 23
# speedup vs baseline: 1.4231x; 1.4231x over previous
"""GCN layer (message passing) on 8 Trainium2 NeuronCores.

out = relu(((D^-1/2 A D^-1/2) X) @ W.T) + X

Strategy (dst-sharded graph partitioning, bf16 gather table):
  - Destination nodes sharded across 8 cores (12500 each); every core holds
    the full feature table and computes its 12500 output rows; host concats.
  - Device prologue: cast the f32 feature table to a bf16 DRAM table h with
    the pre-norm D^-1/2 folded in (h[n] = norm[n] * x[n]); partition p casts
    16 consecutive rows per iteration so DMA descriptors stay contiguous.
    The post-norm norm[dst] is folded into the final ReLU's per-partition
    scale, so the one-hot scatter matrices are pure 0/1.
  - Main loop over PAIRS of dst tiles (2x128 nodes): 4 dma_gather calls per
    pair (one per src bucket of 25088 nodes, int16 indices) pull the edge
    source rows as bf16 into a shared X [128 slots, Cp*128]. Pairing halves
    the SWDGE DMA count so the tile framework's 8 DMASW completion-sem
    lanes recycle every ~4 tiles instead of 2 - the lane-reuse wait (full
    DMA completion of the gather 8 back) stops gating the Pool engine.
    Edges are sorted (pair, bucket, tile, src); the chunk straddling the
    two tiles' boundary is fed to BOTH tiles' matmuls, with the per-tile
    one-hot ld arrays set to -1 for the other tile's slots.
  - Per tile: the one-hot S [128, nct*128] bf16 is built in ONE wide DVE
    tensor_tensor (iota broadcast along chunks, ld broadcast along the 128
    lane dim, via stride-0 APs), then nct bf16 matmuls accumulate zT[i,d]
    in PSUM f32, y = relu(norm_dst * (zT.T @ W.T)) on ACT, residual on DVE.
  - Unwritten X slots (cross-core count spread + chunk padding) are
    memzeroed so NaN garbage can't poison the 0-weighted matmul lanes.
    Idx streams are padded with 0 (gathers bucket row 0; killed by ld=-1).
    (-1 idx padding, which the gather ucode strips, crashes the device.)
"""

import math

import numpy as np

import concourse.bacc as bacc
import concourse.mybir as mybir
from concourse.bass import AP
from concourse.bass_utils import run_bass_kernel_spmd
from concourse.tile import TileContext

P = 128
N_CORES = 8
NB = 4
B = 25088  # bucket size (multiple of 128, int16-indexable)
NPAD = NB * B  # padded node count 100352
CAST_G = 16  # rows per partition per cast iteration
N_NODES = 100000
SINGLE_PACKET = False


def _prepare(features, W, edge_src, edge_dst, n_cores=N_CORES):
    features = np.asarray(features, dtype=np.float32)
    W = np.asarray(W, dtype=np.float32)
    edge_src = np.asarray(edge_src, dtype=np.int32)
    edge_dst = np.asarray(edge_dst, dtype=np.int32)

    n_nodes, d = features.shape
    assert d == P and n_nodes == N_NODES
    npc = n_nodes // n_cores
    n_tiles = math.ceil(npc / P)
    assert n_tiles % 2 == 0
    n_pairs = n_tiles // 2
    rows_last = npc - (n_tiles - 1) * P

    degs = np.bincount(edge_dst, minlength=n_nodes).astype(np.float32)
    norm = 1.0 / np.sqrt(np.maximum(degs, 1.0), dtype=np.float32)
    norm_pad = np.zeros(NPAD, np.float32)
    norm_pad[:n_nodes] = norm

    featspad = np.zeros((NPAD, P), np.float32)
    featspad[:n_nodes] = features

    # normP[p, j*CAST_G + g] = norm[j*128*CAST_G + p*CAST_G + g]
    n_cast_cols = NPAD // P  # 784
    normP = norm_pad.reshape(n_cast_cols // CAST_G, P, CAST_G)
    normP = np.ascontiguousarray(normP.transpose(1, 0, 2).reshape(P, n_cast_cols))

    core_of = edge_dst // npc

    # per-core sorted edge lists, (pair, bucket) counts, even-tile splits
    per_core = []
    counts_pb = np.zeros((n_cores, n_pairs, NB), np.int64)
    counts_even = np.zeros((n_cores, n_pairs, NB), np.int64)
    for k in range(n_cores):
        sel = np.flatnonzero(core_of == k)
        src_k = edge_src[sel]
        ldst = edge_dst[sel] - k * npc
        tile_of = ldst // P
        pair_of = tile_of // 2
        bucket = src_k // B
        order = np.lexsort((src_k, tile_of, bucket, pair_of))
        sel = sel[order]
        t_s = tile_of[order]
        gid = pair_of[order] * NB + bucket[order]
        counts_pb[k] = np.bincount(gid, minlength=n_pairs * NB).reshape(n_pairs, NB)
        counts_even[k] = np.bincount(
            gid[t_s % 2 == 0], minlength=n_pairs * NB
        ).reshape(n_pairs, NB)
        per_core.append((sel, gid, t_s, (ldst[order] % P).astype(np.float32)))

    n_pb = counts_pb.max(axis=0)  # static gather sizes [n_pairs, NB]
    assert n_pb.sum(axis=1).min() > 0
    ct_pb = (n_pb + P - 1) // P  # chunks per (pair, bucket)
    C_p = ct_pb.sum(axis=1)
    icols_pb = (n_pb + 15) // 16
    icols_p = icols_pb.max(axis=1)  # banded layout: buckets share columns

    chunk_off_in_pair = np.cumsum(ct_pb, axis=1) - ct_pb
    icol_off_pair = np.concatenate([[0], np.cumsum(icols_p)])[:-1]
    total_icols = int(icols_p.sum())

    # per-tile chunk ranges within the pair's chunk space (static = cross-core
    # envelope): even tile owns bucket chunks [0, e_end); odd [o_start, cb)
    s_max = counts_even.max(axis=0)
    s_min = counts_even.min(axis=0)
    e_end = (s_max + P - 1) // P  # [n_pairs, NB]
    e_end = np.minimum(e_end, ct_pb)
    o_start = s_min // P
    o_start = np.minimum(o_start, ct_pb)  # empty-odd guard
    nct_even = e_end.sum(axis=1)
    nct_odd = (ct_pb - o_start).sum(axis=1)
    nct_t = np.zeros(n_tiles, np.int64)
    nct_t[0::2] = nct_even
    nct_t[1::2] = nct_odd
    assert nct_t.min() > 0
    ld_col_off = np.concatenate([[0], np.cumsum(nct_t)])[:-1]
    total_C = int(nct_t.sum())

    # chunk lists per tile: pair-chunk index for each S column
    chunk_lists = []
    for t in range(n_tiles):
        p, half = t // 2, t % 2
        lst = []
        for b in range(NB):
            co = int(chunk_off_in_pair[p, b])
            if half == 0:
                lst.extend(range(co, co + int(e_end[p, b])))
            else:
                lst.extend(range(co + int(o_start[p, b]), co + int(ct_pb[p, b])))
        chunk_lists.append(lst)
        assert len(lst) == nct_t[t]

    layout = dict(
        npc=npc,
        n_tiles=n_tiles,
        n_pairs=n_pairs,
        rows_last=rows_last,
        n_pb=n_pb,
        ct_pb=ct_pb,
        C_p=C_p,
        icols_pb=icols_pb,
        icols_p=icols_p,
        chunk_off_in_pair=chunk_off_in_pair,
        icol_off_pair=icol_off_pair,
        total_icols=total_icols,
        nct_t=nct_t,
        ld_col_off=ld_col_off,
        total_C=total_C,
        chunk_lists=chunk_lists,
        n_cast_cols=n_cast_cols,
    )

    ecol_off = np.concatenate(
        [np.zeros((n_pairs, 1), np.int64), np.cumsum(e_end, axis=1)[:, :-1]], axis=1
    )
    ocol_off = np.concatenate(
        [np.zeros((n_pairs, 1), np.int64), np.cumsum(ct_pb - o_start, axis=1)[:, :-1]],
        axis=1,
    )

    in_maps = []
    wt = np.ascontiguousarray(W.T)
    iotam = np.tile(np.arange(P, dtype=np.float32), (P, 1))
    for k in range(n_cores):
        sel, gid, t_s, ld_sorted = per_core[k]
        group_start = np.zeros(n_pairs * NB, np.int64)
        cnts = counts_pb[k].reshape(-1)
        group_start[1:] = np.cumsum(cnts)[:-1]
        pos = np.arange(len(sel)) - group_start[gid]
        p_of = gid // NB
        b_of = gid % NB

        # pad with 0 (gathers bucket row 0; killed by ld=-1 in S).
        # banded: bucket b's stream lives in partitions [32b, 32b+32)
        # (queue b's Q7 core pair), replicated twice within the band.
        idx16 = np.zeros((NB, 16, total_icols), np.int16)
        icol = icol_off_pair[p_of] + pos // 16
        idx16[b_of, pos % 16, icol] = (edge_src[sel] - b_of * B).astype(np.int16)
        idxm = np.concatenate([np.tile(idx16[b], (2, 1)) for b in range(NB)], axis=0)

        # ld array [128, total_C]: tile t's columns are its chunk list; an
        # edge of tile t in pair-chunk (relative) c lands at the column where
        # t's list contains c (straddle chunks appear in both tiles' lists;
        # each edge is written only into its own tile's column)
        import ml_dtypes
        ldm = np.full((P, total_C), -1.0, np.float32)
        rel_chunk = pos // P
        half = t_s % 2
        col_even = ld_col_off[2 * p_of] + ecol_off[p_of, b_of] + rel_chunk
        col_odd = (
            ld_col_off[2 * p_of + 1]
            + ocol_off[p_of, b_of]
            + rel_chunk
            - o_start[p_of, b_of]
        )
        col = np.where(half == 0, col_even, col_odd)
        assert (rel_chunk[half == 0] < e_end[p_of, b_of][half == 0]).all()
        assert (rel_chunk[half == 1] >= o_start[p_of, b_of][half == 1]).all()
        ldm[pos % P, col] = ld_sorted
        ldm = ldm.astype(ml_dtypes.bfloat16)

        # normcol[p, t] = norm[k*npc + t*128 + p] (own dst rows)
        nslice = np.zeros(n_tiles * P, np.float32)
        nslice[:npc] = norm[k * npc : (k + 1) * npc]
        normcol = np.ascontiguousarray(nslice.reshape(n_tiles, P).T)

        in_maps.append(
            {
                "featspad": featspad,
                "idxm": np.ascontiguousarray(idxm),
                "ldm": np.ascontiguousarray(ldm),
                "wt": wt,
                "iotam": iotam,
                "normP": normP,
                "normcol": normcol,
                "resid": np.ascontiguousarray(features[k * npc : (k + 1) * npc]),
            }
        )
    return in_maps, layout


def _build_program(layout):
    f32 = mybir.dt.float32
    bf16 = mybir.dt.bfloat16
    fp8 = mybir.dt.float8e4
    i16 = mybir.dt.int16
    npc = layout["npc"]
    n_tiles = layout["n_tiles"]
    n_pairs = layout["n_pairs"]
    rows_last = layout["rows_last"]
    n_pb = layout["n_pb"]
    ct_pb = layout["ct_pb"]
    C_p = layout["C_p"]
    icols_pb = layout["icols_pb"]
    icols_p = layout["icols_p"]
    chunk_off_in_pair = layout["chunk_off_in_pair"]
    icol_off_pair = layout["icol_off_pair"]
    nct_t = layout["nct_t"]
    ld_col_off = layout["ld_col_off"]
    chunk_lists = layout["chunk_lists"]
    n_cast_cols = layout["n_cast_cols"]
    split_p = chunk_off_in_pair[:, 2]  # first chunk of bucket 2 per pair
    CmaxA = int(split_p.max())
    CmaxB = int((C_p - split_p).max())
    nct_max = int(nct_t.max())

    nc = bacc.Bacc(num_swdge_queues=4, dynamic_dma_scratch_size=32768)
    featspad = nc.declare_dram_parameter("featspad", [NPAD, P], f32, isOutput=False)
    idxm = nc.declare_dram_parameter(
        "idxm", [P, layout["total_icols"]], i16, isOutput=False
    )
    ldm = nc.declare_dram_parameter(
        "ldm", [P, layout["total_C"]], bf16, isOutput=False
    )
    wt = nc.declare_dram_parameter("wt", [P, P], f32, isOutput=False)
    iotam = nc.declare_dram_parameter("iotam", [P, P], f32, isOutput=False)
    normP = nc.declare_dram_parameter("normP", [P, n_cast_cols], f32, isOutput=False)
    normcol = nc.declare_dram_parameter("normcol", [P, n_tiles], f32, isOutput=False)
    resid = nc.declare_dram_parameter("resid", [npc, P], f32, isOutput=False)
    out = nc.declare_dram_parameter("out", [npc, P], bf16, isOutput=True)

    with TileContext(nc) as tc:
        with (
            tc.tile_pool(name="const", bufs=1) as constp,
            tc.tile_pool(name="hdram", bufs=1, space="DRAM") as hp,
            tc.tile_pool(name="cast", bufs=4) as castp,
            tc.tile_pool(name="meta", bufs=3) as metap,
            tc.tile_pool(name="x", bufs=4) as xp,
            tc.tile_pool(name="s", bufs=3) as sp,
            tc.tile_pool(name="zps", bufs=2, space="PSUM") as zpsp,
            tc.tile_pool(name="yps", bufs=2, space="PSUM") as ypsp,
            tc.tile_pool(name="post", bufs=3) as postp,
        ):
            wt_f = constp.tile([P, P], f32)
            nc.sync.dma_start(out=wt_f[:], in_=wt[:, :])
            wt_bf = constp.tile([P, P], bf16)
            nc.scalar.copy(out=wt_bf[:], in_=wt_f[:])
            iota_f = constp.tile([P, P], f32)
            nc.sync.dma_start(out=iota_f[:], in_=iotam[:, :])
            iota_bf = constp.tile([P, P], bf16)
            nc.scalar.copy(out=iota_bf[:], in_=iota_f[:])
            normP_sb = constp.tile([P, n_cast_cols], f32)
            nc.sync.dma_start(out=normP_sb[:], in_=normP[:, :])
            normcol_sb = constp.tile([P, n_tiles], f32)
            nc.sync.dma_start(out=normcol_sb[:], in_=normcol[:, :])

            hfull = hp.tile([NPAD, P], bf16)

            # pre-zero the X pools' rotation slots once: later pairs' unwritten
            # slots then hold zeros or stale finite bf16 from 4 pairs back,
            # which the 0-weighted (ld=-1) one-hot lanes annihilate - NaN-free
            # without per-bucket memzeros on the gather critical path.
            for _ in range(4):
                Xa_zero = xp.tile([P, CmaxA * P], bf16, tag="XA")
                nc.scalar.memzero(Xa_zero[:])
                Xb_zero = xp.tile([P, CmaxB * P], bf16, tag="XB")
                nc.scalar.memzero(Xb_zero[:])

            # ---- prologue: h = norm * x, f32 -> bf16, 2048 rows/iter ----
            for j in range(n_cast_cols // CAST_G):
                r0 = j * P * CAST_G
                cin = castp.tile([P, CAST_G * P], f32, tag="cin")
                nc.scalar.dma_start(
                    out=cin[:].rearrange("p (g d) -> p g d", d=P),
                    in_=featspad[r0 : r0 + P * CAST_G, :].rearrange(
                        "(p g) d -> p g d", g=CAST_G
                    ),
                )
                cout = castp.tile([P, CAST_G * P], bf16, tag="cout")
                nrm = normP_sb[:, j * CAST_G : (j + 1) * CAST_G]
                nc.vector.scalar_tensor_tensor(
                    out=cout[:].rearrange("p (g d) -> p g d", d=P),
                    in0=cin[:].rearrange("p (g d) -> p g d", d=P),
                    scalar=1.0,
                    in1=AP(nrm.tensor, nrm.offset, [nrm.ap[0], [1, CAST_G], [0, P]]),
                    op0=mybir.AluOpType.mult,
                    op1=mybir.AluOpType.mult,
                )
                nc.scalar.dma_start(
                    out=hfull[r0 : r0 + P * CAST_G, :].rearrange(
                        "(p g) d -> p g d", g=CAST_G
                    ),
                    in_=cout[:].rearrange("p (g d) -> p g d", d=P),
                )

            # ---- main loop over pairs of dst tiles ----
            for p in range(n_pairs):
                icols = int(icols_p[p])
                mt_i = metap.tile([P, max(icols, 1)], i16, tag="mi")
                ic0 = int(icol_off_pair[p])
                nc.sync.dma_start(out=mt_i[:, :icols], in_=idxm[:, ic0 : ic0 + icols])

                sp_c = int(split_p[p])
                X_a = xp.tile([P, CmaxA * P], bf16, tag="XA")
                X_b = xp.tile([P, CmaxB * P], bf16, tag="XB")
                for b in range(NB):
                    n_idx = int(n_pb[p, b])
                    if n_idx == 0:
                        continue
                    co = int(chunk_off_in_pair[p, b])
                    cb = int(ct_pb[p, b])
                    io = 0
                    icb = int(icols_pb[p, b])
                    Xh, ch = (X_a, co) if b < 2 else (X_b, co - sp_c)
                    nc.gpsimd.dma_gather(
                        out_ap=Xh[:, ch * P : (ch + cb) * P].rearrange(
                            "p (c e) -> p c e", e=P
                        ),
                        in_ap=hfull[b * B : (b + 1) * B, :],
                        idxs_ap=mt_i[:, io : io + icb],
                        num_idxs=n_idx,
                        num_idxs_reg=n_idx,
                        elem_size=P,
                        single_packet=SINGLE_PACKET,
                        queue_num=b % 4,
                    )

                for half in range(2):
                    t = 2 * p + half
                    nct = int(nct_t[t])
                    lc0 = int(ld_col_off[t])
                    ld_bf = metap.tile([P, nct_max], bf16, tag="mldb")
                    nc.sync.dma_start(out=ld_bf[:, :nct], in_=ldm[:, lc0 : lc0 + nct])

                    # wide 0/1 one-hot: S[e, (j,d)] = (iota[d] == ld[e,j])
                    S = sp.tile([P, nct_max * P], fp8, tag="S")
                    nc.vector.tensor_tensor(
                        out=S[:, : nct * P].rearrange("p (c d) -> p c d", d=P),
                        in0=AP(
                            iota_bf.tensor,
                            iota_bf.offset,
                            [iota_bf.ap[0], [0, nct], [1, P]],
                        ),
                        in1=AP(
                            ld_bf.tensor, ld_bf.offset, [ld_bf.ap[0], [1, nct], [0, P]]
                        ),
                        op=mybir.AluOpType.is_equal,
                    )

                    z_ps = zpsp.tile([P, P], f32)
                    for j, c in enumerate(chunk_lists[t]):
                        # zT[i, d] += X_c[e, i].T @ S_j[e, d]
                        ch2 = c if c < sp_c else c - sp_c
                        Xh2 = X_a if c < sp_c else X_b
                        nc.tensor.matmul(
                            out=z_ps[:],
                            lhsT=Xh2[:, ch2 * P : (ch2 + 1) * P],
                            rhs=S[:, j * P : (j + 1) * P],
                            start=(j == 0),
                            stop=(j == nct - 1),
                        )

                    zT_bf = postp.tile([P, P], bf16, tag="zT")
                    nc.scalar.copy(out=zT_bf[:], in_=z_ps[:])
                    y_ps = ypsp.tile([P, P], f32)
                    # y[d, o] = zT[i, d].T @ wt[i, o]
                    nc.tensor.matmul(
                        out=y_ps[:], lhsT=zT_bf[:], rhs=wt_bf[:], start=True, stop=True
                    )

                    rows = P if t < n_tiles - 1 else rows_last
                    y_sb = postp.tile([P, P], f32, tag="y")
                    # relu(norm_dst * y): post-norm folded into the scale
                    nc.scalar.activation(
                        out=y_sb[:],
                        in_=y_ps[:],
                        func=mybir.ActivationFunctionType.Relu,
                        scale=normcol_sb[:, t : t + 1],
                    )
                    res_sb = postp.tile([P, P], f32, tag="res")
                    nc.sync.dma_start(
                        out=res_sb[:rows], in_=resid[t * P : t * P + rows, :]
                    )
                    o_sb = postp.tile([P, P], bf16, tag="o")
                    nc.vector.tensor_add(
                        out=o_sb[:rows], in0=y_sb[:rows], in1=res_sb[:rows]
                    )
                    nc.sync.dma_start(
                        out=out[t * P : t * P + rows, :], in_=o_sb[:rows]
                    )
    nc.finalize()
    return nc


def _run(features, W, edge_src, edge_dst, trace=False, **spmd_kwargs):
    in_maps, layout = _prepare(features, W, edge_src, edge_dst)
    nc = _build_program(layout)
    br = run_bass_kernel_spmd(
        nc, in_maps, core_ids=list(range(N_CORES)), trace=trace, **spmd_kwargs
    )
    outs = [np.asarray(r["out"]).astype(np.float32) for r in br.results]
    full = np.concatenate(outs, axis=0)
    return full, br


def kernel(features, W, edge_src, edge_dst):
    out, _ = _run(features, W, edge_src, edge_dst, trace=False)
    return out
